# revision 1
# baseline (speedup 1.0000x reference)
"""GAT (2-layer, PyG-style) on 8 Trainium2 NeuronCores.

Strategy (per sharding hint): partition destination nodes across the 8 cores.
Each core:
  dense phase: h_ext = x_shard @ [W1 | W1@A_src | W1@A_dst]  (PE), then
               AllGather the [h | a_src-logit] table so every core can gather
               arbitrary source rows.
  edge phase:  per 128-dst block, dma_gather source rows (sorted-by-dst edge
               tiles), compute exp(leaky_relu(as[src]+ad[dst])) on DVE/ACT
               (no max-subtraction needed: logits are O(1)), and use one-hot
               S matrices on the TensorEngine to do the segment softmax-sum
               and weighted aggregation in PSUM. Normalization by the segment
               sum happens once per 128-dst block.
Layer 2 repeats the same machinery with 1 head / 64 channels.
"""
import sys
sys.path.insert(0, "/opt/trn_rl_repo")

import numpy as np
import concourse.bass as bass
import concourse.bacc as bacc
import concourse.mybir as mybir
from concourse.tile import TileContext
from concourse.bass_utils import run_bass_kernel_spmd

F32 = mybir.dt.float32
I16 = mybir.dt.int16

P = 128
NCORES = 8
LEAKY = 0.2
EPS = 1e-16


class Cfg:
    def __init__(self, N, E, IN_CH=256, HID=256, OUT_CH=64, H1=8):
        self.N, self.E = N, E
        self.IN_CH, self.HID, self.OUT_CH, self.H1 = IN_CH, HID, OUT_CH, H1
        self.C1 = HID // H1
        self.ND = N // NCORES                    # dst nodes per core
        self.NB = (self.ND + P - 1) // P         # dst blocks per core
        self.NPAD = self.NB * P                  # padded shard rows
        self.R = NCORES * self.NPAD              # global table rows
        # half split for int16 gather indices
        self.HALF = (self.R // 2 + P - 1) // P * P
        assert self.HALF < 32768 and (self.R - self.HALF) < 32768
        self.TW1 = 320                           # table1 row width (264 used)
        self.U1 = HID + H1                       # 264
        self.TW2 = 128                           # table2 row width (66 used); 512B rows
        self.U2 = OUT_CH + 2                     # 66


CFG_FULL = Cfg(N=50000, E=800000)


# ---------------------------------------------------------------- host side
def _node_row(cfg, n):
    return (n // cfg.ND) * cfg.NPAD + (n % cfg.ND)


def preprocess_graph(cfg, edge_index):
    """Return per-core wrapped int16 gather indices + dstloc arrays + shared
    per-block tile counts (TA, TB)."""
    src = np.concatenate([edge_index[0], np.arange(cfg.N, dtype=np.int64)])
    dst = np.concatenate([edge_index[1], np.arange(cfg.N, dtype=np.int64)])
    r_src = _node_row(cfg, src)

    core = dst // cfg.ND
    dst_local = dst - core * cfg.ND
    blk = dst_local // P
    dloc = dst_local % P
    in_a = r_src < cfg.HALF

    # bucket edges by (core, block, half)
    NB = cfg.NB
    counts = np.zeros((NCORES, NB, 2), dtype=np.int64)
    np.add.at(counts, (core, blk, (~in_a).astype(np.int64)), 1)
    TA = np.maximum(1, (counts[:, :, 0].max(0) + P - 1) // P)
    TB = np.maximum(1, (counts[:, :, 1].max(0) + P - 1) // P)

    order = np.lexsort((in_a * -1, blk, core))  # group by core, block, half(A first)
    src_s, rsrc_s, core_s, blk_s, dloc_s, ina_s = (
        src[order], r_src[order], core[order], blk[order], dloc[order], in_a[order])

    idx16 = []   # per core: [128, 8 * sum(TA+TB)] int16
    dlocf = []   # per core: [128, sum(TA+TB)] float32
    Tsum = int((TA + TB).sum())
    for c in range(NCORES):
        iw = np.zeros((P, 8 * Tsum), dtype=np.int16)
        dw = np.full((P, Tsum), -1.0, dtype=np.float32)
        csel = core_s == c
        col0 = 0
        for b in range(NB):
            bsel = csel & (blk_s == b)
            for half, T in ((0, int(TA[b])), (1, int(TB[b]))):
                hsel = bsel & (ina_s == (half == 0))
                rr = rsrc_s[hsel] - (0 if half == 0 else cfg.HALF)
                dd = dloc_s[hsel]
                S = T * P
                assert len(rr) <= S
                idx = np.zeros(S, dtype=np.int16)
                idx[: len(rr)] = rr.astype(np.int16)
                dl = np.full(S, -1.0, dtype=np.float32)
                dl[: len(dd)] = dd.astype(np.float32)
                # wrapped idx: i -> [i%16, i//16], replicated to 128 partitions
                w = idx.reshape(S // 16, 16).T
                iw[:, 8 * col0: 8 * col0 + S // 16] = np.tile(w, (8, 1))
                dw[:, col0: col0 + T] = dl.reshape(T, P).T
                col0 += T
        assert col0 == Tsum
        idx16.append(iw)
        dlocf.append(dw)
    return idx16, dlocf, TA.astype(int).tolist(), TB.astype(int).tolist()


def make_weights(cfg, W1, att_src1, att_dst1, W2, att_src2, att_dst2):
    H1, C1 = cfg.H1, cfg.C1
    A1s = np.zeros((cfg.HID, H1), dtype=np.float64)
    A1s[np.arange(cfg.HID), np.arange(cfg.HID) // C1] = att_src1.ravel()
    A1d = np.zeros((cfg.HID, H1), dtype=np.float64)
    A1d[np.arange(cfg.HID), np.arange(cfg.HID) // C1] = att_dst1.ravel()
    W1f = np.concatenate([W1, W1 @ A1s, W1 @ A1d], axis=1).astype(np.float32)  # [IN, 272]
    W2f = np.concatenate([W2, W2 @ att_src2.T, W2 @ att_dst2.T], axis=1).astype(np.float32)  # [HID, 66]
    return W1f, W2f


# ---------------------------------------------------------------- device side
GATHER_CHUNK_TILES = 6   # 768 idxs < 1024-descriptor SWDGE ring capacity


def _gather_chunks(TA, TB, table, HALF, R):
    """Yield (tile_col_start, tile_col_end, table_slice) chunks, each at most
    GATHER_CHUNK_TILES tiles, never straddling the A/B half boundary."""
    out = []
    for lo, hi, tbl in ((0, TA, table[0:HALF, :]), (TA, TA + TB, table[HALF:R, :])):
        c = lo
        while c < hi:
            e = min(c + GATHER_CHUNK_TILES, hi)
            out.append((c, e, tbl))
            c = e
    return out

def build_kernel(cfg, TA, TB, Tsum, phases="abc"):
    nc = bacc.Bacc("TRN2", target_bir_lowering=False, debug=False,
                   num_devices=NCORES)
    IN, HID, OUT, H1, C1 = cfg.IN_CH, cfg.HID, cfg.OUT_CH, cfg.H1, cfg.C1
    U1, U2, TW1, TW2 = cfg.U1, cfg.U2, cfg.TW1, cfg.TW2
    NB, NPAD, R, HALF = cfg.NB, cfg.NPAD, cfg.R, cfg.HALF
    KI = IN // P   # k-chunks for layer-1 dense
    KH = HID // P  # k-chunks for layer-2 dense

    X = nc.declare_dram_parameter("X", [NPAD, IN], F32, isOutput=False)
    W1F = nc.declare_dram_parameter("W1F", [IN, U1 + H1], F32, isOutput=False)
    W2F = nc.declare_dram_parameter("W2F", [HID, U2], F32, isOutput=False)
    IDX = nc.declare_dram_parameter("IDX", [P, 8 * Tsum], I16, isOutput=False)
    DLOC = nc.declare_dram_parameter("DLOC", [P, Tsum], F32, isOutput=False)
    IOTA = nc.declare_dram_parameter("IOTA", [P, P], F32, isOutput=False)
    IOTAC = nc.declare_dram_parameter("IOTAC", [P, 1], F32, isOutput=False)
    IDENT = nc.declare_dram_parameter("IDENT", [P, P], F32, isOutput=False)
    B1R = nc.declare_dram_parameter("B1R", [P, HID], F32, isOutput=False)
    B2R = nc.declare_dram_parameter("B2R", [P, OUT], F32, isOutput=False)
    OUTT = nc.declare_dram_parameter("OUTT", [cfg.ND, OUT], F32, isOutput=True)

    with TileContext(nc, num_cores=NCORES) as tc:
        with (
            tc.tile_pool(name="const", bufs=1) as cpool,
            tc.tile_pool(name="dram", bufs=1, space="DRAM") as dram,
            tc.tile_pool(name="psum", bufs=2, space="PSUM") as psum,
        ):
            # resident constants
            iota_sb = cpool.tile([P, P], F32)
            nc.sync.dma_start(out=iota_sb[:], in_=IOTA[:, :])
            iotac_sb = cpool.tile([P, 1], F32)
            nc.sync.dma_start(out=iotac_sb[:], in_=IOTAC[:, :])
            ident_sb = cpool.tile([P, P], F32)
            nc.sync.dma_start(out=ident_sb[:], in_=IDENT[:, :])
            b1_sb = cpool.tile([P, HID], F32)
            nc.sync.dma_start(out=b1_sb[:], in_=B1R[:, :])
            b2_sb = cpool.tile([P, OUT], F32)
            nc.sync.dma_start(out=b2_sb[:], in_=B2R[:, :])
            w1f_sb = cpool.tile([P, KI, U1 + H1], F32)
            for k in range(KI):
                nc.sync.dma_start(out=w1f_sb[:, k, :], in_=W1F[k * P:(k + 1) * P, :])
            w2f_sb = cpool.tile([P, KH, U2], F32)
            for k in range(KH):
                nc.sync.dma_start(out=w2f_sb[:, k, :], in_=W2F[k * P:(k + 1) * P, :])

            shard1 = dram.tile([NPAD, TW1], F32)
            table1 = dram.tile([R, TW1], F32, addr_space="Shared")
            shard2 = dram.tile([NPAD, TW2], F32)
            table2 = dram.tile([R, TW2], F32, addr_space="Shared")
            ad1_sh = dram.tile([NPAD, H1], F32)

            # ---------------- phase A: h_ext = x @ W1F, write table1 shard
            with (
                tc.tile_pool(name="pa_sb", bufs=3) as sb,
            ):
                ps, pst = psum, psum
                for rb in range(NB):
                    xt = sb.tile([P, IN], F32, tag="xt")
                    nc.sync.dma_start(out=xt[:], in_=X[rb * P:(rb + 1) * P, :])
                    xT = sb.tile([P, KI, P], F32, tag="xT")
                    for k in range(KI):
                        ptr = pst.tile([P, P], F32, tag="ptr")
                        nc.tensor.transpose(out=ptr[:], in_=xt[:, k * P:(k + 1) * P],
                                            identity=ident_sb[:])
                        nc.scalar.copy(out=xT[:, k, :], in_=ptr[:])
                    ph = ps.tile([P, U1 + H1], F32, tag="pm")
                    for k in range(KI):
                        nc.tensor.matmul(out=ph[:], lhsT=xT[:, k, :],
                                         rhs=w1f_sb[:, k, :],
                                         start=(k == 0), stop=(k == KI - 1))
                    hrow = sb.tile([P, U1 + H1], F32, tag="hrow")
                    nc.scalar.copy(out=hrow[:], in_=ph[:])
                    nc.sync.dma_start(out=shard1[rb * P:(rb + 1) * P, 0:U1],
                                      in_=hrow[:, 0:U1])
                    nc.sync.dma_start(out=ad1_sh[rb * P:(rb + 1) * P, :],
                                      in_=hrow[:, U1:U1 + H1])

            nc.gpsimd.collective_compute(
                "AllGather", mybir.AluOpType.bypass,
                replica_groups=[list(range(NCORES))],
                ins=[shard1[:, :].opt()], outs=[table1[:, :].opt()])

            # ---------------- phase B: layer-1 edge aggregation
            if "b" not in phases:
                dummy = cpool.tile([P, 1], F32, name="dummy")
                nc.sync.dma_start(out=dummy[:], in_=table1[0:P, 0:1])
                nc.sync.dma_start(out=OUTT[0:P, 0:1], in_=dummy[:])
                return nc
            with (
                tc.tile_pool(name="pb_he", bufs=2) as p_he,
                tc.tile_pool(name="pb_sb", bufs=2) as sb,
                tc.tile_pool(name="pb_small", bufs=3) as sm,
            ):
                ps_main = ps_tr = ps_ad = psum
                icol = 0
                for b in range(NB):
                    T = TA[b] + TB[b]
                    tidx = sm.tile([P, 8 * T], I16, tag="tidx")
                    nc.sync.dma_start(out=tidx[:], in_=IDX[:, 8 * icol: 8 * (icol + T)])
                    dloc = sm.tile([P, T], F32, tag="dloc")
                    nc.sync.dma_start(out=dloc[:], in_=DLOC[:, icol: icol + T])
                    adb = sm.tile([P, H1], F32, tag="adb")
                    nc.sync.dma_start(out=adb[:], in_=ad1_sh[b * P:(b + 1) * P, :])

                    he = p_he.tile([P, T, TW1], F32, tag="he")
                    for c0, c1, tbl in _gather_chunks(TA[b], TB[b], table1, HALF, R):
                        nc.gpsimd.dma_gather(
                            he[:, c0:c1, :], tbl, tidx[:, 8 * c0:8 * c1],
                            num_idxs=(c1 - c0) * P, num_idxs_reg=(c1 - c0) * P,
                            elem_size=TW1)

                    # one-hot S (edges x dst) for every tile in one op
                    S = sb.tile([P, T, P], F32, tag="S")
                    nc.vector.tensor_tensor(
                        out=S[:], in0=iota_sb[:].unsqueeze(1).to_broadcast([P, T, P]),
                        in1=dloc[:].unsqueeze(2).to_broadcast([P, T, P]),
                        op=mybir.AluOpType.is_equal)

                    # ad1 per edge: St = S^T via PE transpose of dloc bcast, then St @ adb
                    pad = ps_ad.tile([P, T * H1], F32, tag="pad")
                    for t in range(T):
                        ptr = ps_tr.tile([P, P], F32, tag="ptr")
                        nc.tensor.transpose(
                            out=ptr[:], in_=dloc[:, t:t + 1].to_broadcast([P, P]),
                            identity=ident_sb[:])
                        St = sm.tile([P, P], F32, tag="St")
                        nc.vector.tensor_scalar(
                            out=St[:], in0=ptr[:], scalar1=iotac_sb[:, 0:1],
                            scalar2=None, op0=mybir.AluOpType.is_equal)
                        nc.tensor.matmul(out=pad[:, t * H1:(t + 1) * H1],
                                         lhsT=St[:], rhs=adb[:],
                                         start=True, stop=True)

                    # exp(leaky(as + ad))
                    sume = sb.tile([P, T * H1], F32, tag="sume")
                    nc.vector.tensor_tensor(
                        out=sume[:].rearrange("p (t h) -> p t h", h=H1),
                        in0=he[:, :, U1 - H1:U1],
                        in1=pad[:].rearrange("p (t h) -> p t h", h=H1),
                        op=mybir.AluOpType.add)
                    lk = sb.tile([P, T * H1], F32, tag="lk")
                    nc.vector.scalar_tensor_tensor(
                        out=lk[:], in0=sume[:], scalar=LEAKY, in1=sume[:],
                        op0=mybir.AluOpType.mult, op1=mybir.AluOpType.max)
                    rhs = sb.tile([P, T, H1 + HID], F32, tag="rhs")
                    nc.scalar.activation(
                        out=rhs[:, :, 0:H1],
                        in_=lk[:].rearrange("p (t h) -> p t h", h=H1),
                        func=mybir.ActivationFunctionType.Exp)
                    # Mw = h * ex (broadcast ex over the 32 chans of each head)
                    nc.vector.tensor_tensor(
                        out=rhs[:, :, H1:].rearrange("p t (h c) -> p t h c", h=H1),
                        in0=he[:, :, 0:HID].rearrange("p t (h c) -> p t h c", h=H1),
                        in1=rhs[:, :, 0:H1].unsqueeze(3).to_broadcast([P, T, H1, C1]),
                        op=mybir.AluOpType.mult)

                    pm = ps_main.tile([P, H1 + HID], F32, tag="pm")
                    for t in range(T):
                        nc.tensor.matmul(out=pm[:], lhsT=S[:, t, :], rhs=rhs[:, t, :],
                                         start=(t == 0), stop=(t == T - 1))

                    # normalize + bias + ELU -> h2 block
                    srec = sm.tile([P, H1], F32, tag="srec")
                    nc.vector.tensor_scalar(
                        out=srec[:], in0=pm[:, 0:H1], scalar1=EPS, scalar2=None,
                        op0=mybir.AluOpType.add)
                    nc.vector.reciprocal(out=srec[:], in_=srec[:])
                    t2 = sb.tile([P, HID], F32, tag="t2")
                    nc.vector.tensor_tensor(
                        out=t2[:].rearrange("p (h c) -> p h c", h=H1),
                        in0=pm[:, H1:].rearrange("p (h c) -> p h c", h=H1),
                        in1=srec[:].unsqueeze(2).to_broadcast([P, H1, C1]),
                        op=mybir.AluOpType.mult)
                    nc.vector.tensor_tensor(out=t2[:], in0=t2[:], in1=b1_sb[:],
                                            op=mybir.AluOpType.add)
                    # elu(x) = max(x,0) + exp(min(x,0)) - 1
                    mm = sb.tile([P, HID], F32, tag="mm")
                    nc.vector.tensor_scalar(out=mm[:], in0=t2[:], scalar1=0.0,
                                            scalar2=None, op0=mybir.AluOpType.min)
                    qq = sb.tile([P, HID], F32, tag="qq")
                    nc.scalar.activation(out=qq[:], in_=mm[:],
                                         func=mybir.ActivationFunctionType.Exp)
                    pp = sb.tile([P, HID], F32, tag="pp")
                    nc.scalar.activation(out=pp[:], in_=t2[:],
                                         func=mybir.ActivationFunctionType.Relu)
                    h2 = sb.tile([P, HID], F32, tag="h2")
                    nc.vector.scalar_tensor_tensor(
                        out=h2[:], in0=qq[:], scalar=-1.0, in1=pp[:],
                        op0=mybir.AluOpType.add, op1=mybir.AluOpType.add)

                    # layer-2 dense for this block: g_ext = h2 @ W2F
                    h2T = sb.tile([P, KH, P], F32, tag="h2T")
                    for k in range(KH):
                        ptr2 = ps_tr.tile([P, P], F32, tag="ptr")
                        nc.tensor.transpose(out=ptr2[:], in_=h2[:, k * P:(k + 1) * P],
                                            identity=ident_sb[:])
                        nc.scalar.copy(out=h2T[:, k, :], in_=ptr2[:])
                    pg = ps_ad.tile([P, U2], F32, tag="pg")
                    for k in range(KH):
                        nc.tensor.matmul(out=pg[:], lhsT=h2T[:, k, :],
                                         rhs=w2f_sb[:, k, :],
                                         start=(k == 0), stop=(k == KH - 1))
                    gr = sb.tile([P, U2], F32, tag="gr")
                    nc.scalar.copy(out=gr[:], in_=pg[:])
                    nc.sync.dma_start(out=shard2[b * P:(b + 1) * P, 0:U2], in_=gr[:])
                    icol += T

            if "g" in phases or "c" in phases:
                nc.gpsimd.collective_compute(
                    "AllGather", mybir.AluOpType.bypass,
                    replica_groups=[list(range(NCORES))],
                    ins=[shard2[:, :].opt()], outs=[table2[:, :].opt()])
            if "c" not in phases:
                dummy2 = cpool.tile([P, 1], F32, name="dummy2")
                src_t = table2 if ("g" in phases) else shard2
                nc.sync.dma_start(out=dummy2[:], in_=src_t[0:P, 0:1])
                nc.sync.dma_start(out=OUTT[0:P, 0:1], in_=dummy2[:])
                return nc

            # ---------------- phase C: layer-2 edge aggregation
            with (
                tc.tile_pool(name="pc_ge", bufs=2) as p_ge,
                tc.tile_pool(name="pc_sb", bufs=2) as sb,
                tc.tile_pool(name="pc_small", bufs=3) as sm,
            ):
                ps_main = ps_tr = ps_ad = psum
                icol = 0
                for b in range(NB):
                    T = TA[b] + TB[b]
                    tidx = sm.tile([P, 8 * T], I16, tag="tidx")
                    nc.sync.dma_start(out=tidx[:], in_=IDX[:, 8 * icol: 8 * (icol + T)])
                    dloc = sm.tile([P, T], F32, tag="dloc")
                    nc.sync.dma_start(out=dloc[:], in_=DLOC[:, icol: icol + T])

                    ge = p_ge.tile([P, T, TW2], F32, tag="ge")
                    for c0, c1, tbl in _gather_chunks(TA[b], TB[b], table2, HALF, R):
                        nc.gpsimd.dma_gather(
                            ge[:, c0:c1, :], tbl, tidx[:, 8 * c0:8 * c1],
                            num_idxs=(c1 - c0) * P, num_idxs_reg=(c1 - c0) * P,
                            elem_size=TW2)

                    S = sb.tile([P, T, P], F32, tag="S")
                    nc.vector.tensor_tensor(
                        out=S[:], in0=iota_sb[:].unsqueeze(1).to_broadcast([P, T, P]),
                        in1=dloc[:].unsqueeze(2).to_broadcast([P, T, P]),
                        op=mybir.AluOpType.is_equal)

                    # ad2 per edge via per-tile S^T @ ad2_blk (shard2 col 65)
                    adb2 = sm.tile([P, 1], F32, tag="adb2")
                    nc.sync.dma_start(out=adb2[:],
                                      in_=shard2[b * P:(b + 1) * P, U2 - 1:U2])
                    pad2 = ps_ad.tile([P, T], F32, tag="pad")
                    for t in range(T):
                        ptr = ps_tr.tile([P, P], F32, tag="ptr")
                        nc.tensor.transpose(
                            out=ptr[:], in_=dloc[:, t:t + 1].to_broadcast([P, P]),
                            identity=ident_sb[:])
                        St = sm.tile([P, P], F32, tag="St")
                        nc.vector.tensor_scalar(
                            out=St[:], in0=ptr[:], scalar1=iotac_sb[:, 0:1],
                            scalar2=None, op0=mybir.AluOpType.is_equal)
                        nc.tensor.matmul(out=pad2[:, t:t + 1],
                                         lhsT=St[:], rhs=adb2[:],
                                         start=True, stop=True)

                    sum2 = sm.tile([P, T], F32, tag="sum2")
                    nc.vector.tensor_tensor(
                        out=sum2[:], in0=ge[:, :, U2 - 2:U2 - 1].squeeze(2),
                        in1=pad2[:],
                        op=mybir.AluOpType.add)
                    lk2 = sm.tile([P, T], F32, tag="lk2")
                    nc.vector.scalar_tensor_tensor(
                        out=lk2[:], in0=sum2[:], scalar=LEAKY, in1=sum2[:],
                        op0=mybir.AluOpType.mult, op1=mybir.AluOpType.max)
                    rhs2 = sb.tile([P, T, 1 + OUT], F32, tag="rhs2")
                    nc.scalar.activation(out=rhs2[:, :, 0:1],
                                         in_=lk2[:].unsqueeze(2),
                                         func=mybir.ActivationFunctionType.Exp)
                    nc.vector.tensor_tensor(
                        out=rhs2[:, :, 1:],
                        in0=ge[:, :, 0:OUT],
                        in1=rhs2[:, :, 0:1].to_broadcast([P, T, OUT]),
                        op=mybir.AluOpType.mult)

                    pm2 = ps_main.tile([P, 1 + OUT], F32, tag="pm")
                    for t in range(T):
                        nc.tensor.matmul(out=pm2[:], lhsT=S[:, t, :], rhs=rhs2[:, t, :],
                                         start=(t == 0), stop=(t == T - 1))

                    rec2 = sm.tile([P, 1], F32, tag="rec2")
                    nc.vector.tensor_scalar(
                        out=rec2[:], in0=pm2[:, 0:1], scalar1=EPS, scalar2=None,
                        op0=mybir.AluOpType.add)
                    nc.vector.reciprocal(out=rec2[:], in_=rec2[:])
                    ob = sb.tile([P, OUT], F32, tag="ob")
                    nc.vector.scalar_tensor_tensor(
                        out=ob[:], in0=pm2[:, 1:], scalar=rec2[:, 0:1], in1=b2_sb[:],
                        op0=mybir.AluOpType.mult, op1=mybir.AluOpType.add)
                    nrows = min(P, cfg.ND - b * P)
                    nc.sync.dma_start(out=OUTT[b * P: b * P + nrows, :],
                                      in_=ob[0:nrows, :])
                    icol += T
    return nc


# ---------------------------------------------------------------- entry point
def gat_run(cfg, x, edge_index, W1, att_src1, att_dst1, b1, W2, att_src2,
            att_dst2, b2, trace=False):
    x = np.asarray(x, dtype=np.float32)
    edge_index = np.asarray(edge_index)
    W1f, W2f = make_weights(cfg, np.asarray(W1, np.float64),
                            np.asarray(att_src1, np.float64),
                            np.asarray(att_dst1, np.float64),
                            np.asarray(W2, np.float64),
                            np.asarray(att_src2, np.float64),
                            np.asarray(att_dst2, np.float64))
    idx16, dlocf, TA, TB = preprocess_graph(cfg, edge_index.astype(np.int64))
    Tsum = sum(TA) + sum(TB)

    nc = build_kernel(cfg, TA, TB, Tsum)
    nc.finalize()

    iota = np.broadcast_to(np.arange(P, dtype=np.float32), (P, P)).copy()
    iotac = np.arange(P, dtype=np.float32)[:, None].copy()
    ident = np.eye(P, dtype=np.float32)
    b1r = np.broadcast_to(np.asarray(b1, np.float32), (P, cfg.HID)).copy()
    b2r = np.broadcast_to(np.asarray(b2, np.float32), (P, cfg.OUT_CH)).copy()

    in_maps = []
    for c in range(NCORES):
        xs = np.zeros((cfg.NPAD, cfg.IN_CH), dtype=np.float32)
        xs[: cfg.ND] = x[c * cfg.ND:(c + 1) * cfg.ND]
        in_maps.append({
            "X": xs, "W1F": W1f, "W2F": W2f,
            "IDX": idx16[c], "DLOC": dlocf[c],
            "IOTA": iota, "IOTAC": iotac, "IDENT": ident,
            "B1R": b1r, "B2R": b2r,
        })
    res = run_bass_kernel_spmd(nc, in_maps, list(range(NCORES)), trace=trace)
    out = np.concatenate([res.results[c]["OUTT"] for c in range(NCORES)], axis=0)
    return out, res


def kernel(x, edge_index, W1, att_src1, att_dst1, b1, W2, att_src2, att_dst2,
           b2):
    out, _ = gat_run(CFG_FULL, x, edge_index, W1, att_src1, att_dst1, b1, W2,
                     att_src2, att_dst2, b2)
    return out.astype(np.float32)

